# revision 2
# baseline (speedup 1.0000x reference)
"""Causal MHA attention-out kernel for TRN2, head-sharded across 8 NeuronCores.

Reference computation (fp32):
    scores = (q @ k^T) / sqrt(64), causal mask, softmax
    z      = pattern @ v
    out    = sum_h z_h @ W_O[h] + b_O          (residual passed through)

Sharding: 16 heads -> 8 cores x 2 adjacent heads. Each core computes a
partial out (its 2 heads' contribution, both batches); host sums partials.

Pipeline: one global chunk stream over (qc, b, kb). The score tile is one
k-block for BOTH heads ([128, 1024] f32 = 2 PSUM banks, head0 cols 0-511,
head1 cols 512-1023), double-buffered so QK(kb+1) overlaps exp(kb). One ACT
exp call per k-block head-region (leading fully-masked cols trimmed; ACT is
the bottleneck engine at ~79us/iter). PV matmuls trail QK/exp by 2 chunks so
the in-order PE queue never waits on ACT; each head's softmax normalize
(denominator = ones-column folded into v_aug, accumulated in zacc row 64) is
emitted right after that head's last PV, and the previous steps' output
projections are dripped 2 tasks/chunk from a global queue. PSUM budget:
scores 2x2 banks + zacc 3x1 (triple-buffered to absorb the normalize-chain
latency at step boundaries) + proj 1x1 = 8 banks.

HW-measured pitfalls encoded here (do not "simplify" these away):
  - nc.vector.reciprocal is an 8-pass iterative divide: ~75us/iter slower
    end-to-end. reciprocal_approx_fast (~18 bits) is used instead, but it
    must read SBUF — from PSUM it returns garbage on HW (sim disagrees).
  - dma_start_transpose on the Activation HWDGE queue corrupts data on HW;
    all transposes go on the SP queue, in need-order so compute starts
    after the first two and the rest stream behind it.
  - All matmul operands bf16 (f32r measured 4x slower); accumulation f32.
"""

import numpy as np

import concourse.bass as bass
import concourse.mybir as mybir
from concourse import bacc
import concourse.tile as tile
from concourse.bass_utils import run_bass_kernel_spmd

B = 2
S = 2048
D_MODEL = 1024
N_HEADS = 16
D_HEAD = 64
N_CORES = 8
HPC = 2  # heads per core
CW = HPC * D_HEAD  # 128 columns of q/k/v per core
NKB = S // 128  # 16 k-blocks
NQC = S // 512  # 4 q-chunks
INV_SCALE = 1.0 / 8.0  # 1/sqrt(64)
PV_LAG = 2  # chunks of lag between QK/exp and the PV that consumes them

F32 = mybir.dt.float32
MMDT = mybir.dt.bfloat16  # matmul operand dtype: 1 cyc/row on PE

_CACHE = {}


def _build_bass(reps=None):
    nc = bacc.Bacc("TRN2", target_bir_lowering=False)

    q_d = nc.dram_tensor("q", [B, S, CW], MMDT, kind="ExternalInput")
    k_d = nc.dram_tensor("k", [B, S, CW], MMDT, kind="ExternalInput")
    v_d = nc.dram_tensor("v", [B, S, CW], MMDT, kind="ExternalInput")
    wo_d = nc.dram_tensor("wo", [CW, D_MODEL], MMDT, kind="ExternalInput")
    out_d = nc.dram_tensor("out", [B, S, D_MODEL], MMDT, kind="ExternalOutput")

    with tile.TileContext(nc) as tc:
        with (
            tc.tile_pool(name="const", bufs=1) as const_pool,
            tc.tile_pool(name="big", bufs=3) as big_pool,
            tc.tile_pool(name="zsb", bufs=2) as zsb_pool,
            tc.tile_pool(name="stage", bufs=4) as stage_pool,
            tc.tile_pool(name="pat", bufs=8) as pat_pool,
            tc.tile_pool(name="osb", bufs=4) as osb_pool,
            tc.tile_pool(name="ps", bufs=2, space="PSUM") as ps_pool,
        ):
            ones16 = const_pool.tile([128, NKB], F32)
            nc.gpsimd.memset(ones16, 1.0)
            wo_sb = const_pool.tile([CW, D_MODEL], MMDT)
            nc.sync.dma_start(wo_sb, wo_d[:, :])

            import contextlib

            loop_cm = (
                tc.For_i(
                    0,
                    reps,
                    1,
                    hint_engines=(
                        mybir.EngineType.PE,
                        mybir.EngineType.DVE,
                        mybir.EngineType.Activation,
                        mybir.EngineType.Pool,
                        mybir.EngineType.SP,
                    ),
                    staggered_reset=True,
                )
                if reps
                else contextlib.nullcontext()
            )
            with loop_cm:
                env = locals()
                tiles = _emit_loads(nc, env)
                _emit_compute(nc, env, tiles)
    nc.compile()
    return nc


def _emit_loads(nc, env):
    (q_d, k_d, v_d) = (env["q_d"], env["k_d"], env["v_d"])
    big_pool = env["big_pool"]
    ones16 = env["ones16"]

    kTs, qTs, vbigs = [], [], []
    for b in range(B):
        kT = big_pool.tile([128, S], MMDT, tag="kT", name=f"kT{b}")
        qT = big_pool.tile([128, S], MMDT, tag="qT", name=f"qT{b}")
        # v packed per k-block as [v_h0 | ones | v_h1 | ones] (130 cols)
        vbig = big_pool.tile([128, NKB * 130], MMDT, tag="vb", name=f"vb{b}")
        kTs.append(kT); qTs.append(qT); vbigs.append(vbig)

    def _emit_v(b):
        v3 = vbigs[b].rearrange("p (t c) -> p t c", c=130)
        nc.sync.dma_start(
            v3[:, :, 0:64],
            v_d[b].rearrange("(t p) c -> p t c", p=128)[:, :, 0:64],
        )
        nc.scalar.dma_start(
            v3[:, :, 65:129],
            v_d[b].rearrange("(t p) c -> p t c", p=128)[:, :, 64:128],
        )
        nc.gpsimd.tensor_copy(v3[:, :, 64], ones16)
        nc.gpsimd.tensor_copy(v3[:, :, 129], ones16)

    # bf16 xbar DMA transposes, d-major (head0 -> partitions 0-63, head1 ->
    # 64-127). All on the SP HWDGE queue (ACT-queue transposes corrupt data
    # on HW), in need-order: the first compute step starts after the first
    # two, the rest stream behind compute.
    for c4 in range(4):
        for b in range(B):
            for src_, dstT in ((k_d, kTs[b]), (q_d, qTs[b])):
                nc.sync.dma_start_transpose(
                    dstT[:, c4 * 512 : (c4 + 1) * 512],
                    src_[b, c4 * 512 : (c4 + 1) * 512, :],
                )
            if c4 == 0:
                _emit_v(b)
    return kTs, qTs, vbigs


def _emit_compute(nc, env, tiles):
    out_d = env["out_d"]
    (zsb_pool, stage_pool, pat_pool, osb_pool, ps_pool) = (
        env["zsb_pool"], env["stage_pool"], env["pat_pool"], env["osb_pool"],
        env["ps_pool"]
    )
    wo_sb = env["wo_sb"]
    kTs, qTs, vbigs = tiles

    steps = [(qc, b) for qc in range(NQC) for b in range(B)]
    chunks = [(qc, b, kb) for (qc, b) in steps for kb in range(4 * qc + 4)]
    state = {}
    for qc, b in steps:
        state[(qc, b)] = {"zacc": None, "pats": {}, "nkb": 4 * qc + 4}

    proj_queue = []  # (zsb, b, qc, qb, mch)
    osb_tiles = {}
    # final-flush proj scratch rotates across the then-idle PSUM slots so the
    # op-chain doesn't serialize the iteration tail
    flush_slots = [("op", 1), ("z", 3), ("sc", 2)]

    def emit_proj_task(task, slot_tag="op", slot_bufs=1):
        zsb_p, pb, pqc, qb, mch = task
        op = ps_pool.tile([128, 512], F32, tag=slot_tag, bufs=slot_bufs,
                          name=f"op{pb}_{pqc}_{qb}_{mch}")
        nc.tensor.matmul(
            op,
            lhsT=zsb_p[:, qb * 128 : (qb + 1) * 128],
            rhs=wo_sb[:, mch * 512 : (mch + 1) * 512],
            start=True,
            stop=True,
        )
        key = (pb, pqc, qb)
        if key not in osb_tiles:
            osb_tiles[key] = osb_pool.tile(
                [128, D_MODEL], MMDT, tag="osb", name=f"osb{pb}_{pqc}_{qb}"
            )
        osb = osb_tiles[key]
        nc.vector.tensor_copy(osb[:, mch * 512 : (mch + 1) * 512], op)
        if mch == 1:
            r0 = pqc * 512 + qb * 128
            nc.sync.dma_start(out_d[pb, r0 : r0 + 128, :], osb)

    def emit_pv(qc, b, kb):
        st = state[(qc, b)]
        nkb = st["nkb"]
        dd = kb - 4 * qc
        s = 128 * dd if dd > 0 else 0
        vbig = vbigs[b]
        for h in range(HPC):
            nc.tensor.matmul(
                st["zacc"][h][:, s:512],
                lhsT=vbig[:, kb * 130 + 65 * h : kb * 130 + 65 * h + 65],
                rhs=st["pats"][kb][:, 512 * h + s : 512 * (h + 1)],
                start=(kb == 0),
                stop=(kb == nkb - 1),
            )
            if kb == nkb - 1:
                # normalize head h: zT = zT / denom (denom = ones-col of
                # v_aug, accumulated into zacc row 64)
                zacc_h = st["zacc"][h]
                r_sb = stage_pool.tile([1, 512], F32, tag="r",
                                       name=f"r{b}_{qc}_{h}")
                # the custom-DVE approx op reads garbage from PSUM on HW, so
                # stage the denominator row through SBUF first; ~18 correct
                # bits is plenty, and the exact `reciprocal` (8-pass
                # iterative divide) measured ~75us/iter slower end-to-end
                d_sb = stage_pool.tile([1, 512], F32, tag="d",
                                       name=f"d{b}_{qc}_{h}")
                nc.vector.tensor_copy(d_sb, zacc_h[64:65, :])
                nc.vector.reciprocal_approx_fast(r_sb, d_sb)
                rb = stage_pool.tile([64, 512], F32, tag="rb",
                                     name=f"rb{b}_{qc}_{h}")
                nc.gpsimd.partition_broadcast(rb, r_sb)
                nc.vector.tensor_mul(
                    st["zsb"][64 * h : 64 * h + 64, :],
                    zacc_h[0:64, :],
                    rb,
                )
        if kb == nkb - 1:
            zsb = st["zsb"]
            for qb in range(4):
                for mch in range(2):
                    proj_queue.append((zsb, b, qc, qb, mch))

    pv_pending = []
    for ci, (qc, b, kb) in enumerate(chunks):
        st = state[(qc, b)]
        if kb == 0:
            st["zacc"] = [
                ps_pool.tile([65, 512], F32, tag="z", bufs=3,
                             name=f"zacc{b}_{qc}_{h}")
                for h in range(HPC)
            ]
            st["zsb"] = zsb_pool.tile([128, 512], MMDT, tag="zsb",
                                      name=f"zsb{b}_{qc}")
        kT, qT = kTs[b], qTs[b]
        dd = kb - 4 * qc
        s = 128 * dd if dd > 0 else 0
        sc = ps_pool.tile([128, 1024], F32, tag="sc", bufs=2,
                          name=f"sc{b}_{qc}_{kb}")
        # QK^T: scoresT[k, q]; the two heads run concurrently in disjoint PE
        # row halves (tile_position auto-derived from base partition)
        for h in range(HPC):
            nc.tensor.matmul(
                sc[:, 512 * h + s : 512 * (h + 1)],
                lhsT=kT[64 * h : 64 * h + 64, kb * 128 : (kb + 1) * 128],
                rhs=qT[64 * h : 64 * h + 64, qc * 512 + s : (qc + 1) * 512],
                start=True,
                stop=True,
            )
        pat = pat_pool.tile([128, 1024], MMDT, tag="pat",
                            name=f"pat{b}_{qc}_{kb}")
        st["pats"][kb] = pat
        # exp per head-region; on diagonal chunks (s>0) the two regions are
        # disjoint (cols [512, 512+s) are never written or read)
        for e0, e1 in ([(s, 1024)] if s == 0 else [(s, 512), (512 + s, 1024)]):
            nc.scalar.activation(
                pat[:, e0:e1],
                sc[:, e0:e1],
                mybir.ActivationFunctionType.Exp,
                scale=INV_SCALE,
            )
        if dd >= 0:
            # zero the upper triangle of the diagonal 128x128 block
            for h in range(HPC):
                ap = pat[:, 512 * h + s : 512 * h + s + 128]
                nc.gpsimd.affine_select(
                    out=ap,
                    in_=ap,
                    compare_op=mybir.AluOpType.is_ge,
                    fill=0.0,
                    base=0,
                    pattern=[[1, 128]],
                    channel_multiplier=-1,
                )
        pv_pending.append((qc, b, kb))
        if len(pv_pending) > PV_LAG:
            emit_pv(*pv_pending.pop(0))
        for _ in range(2):
            if proj_queue:
                emit_proj_task(proj_queue.pop(0))
    while pv_pending:
        emit_pv(*pv_pending.pop(0))
    i = 0
    while proj_queue:
        tag, bufs_ = flush_slots[i % len(flush_slots)]
        emit_proj_task(proj_queue.pop(0), slot_tag=tag, slot_bufs=bufs_)
        i += 1


def make_in_maps(q, k, v, W_O):
    import ml_dtypes

    bf16 = ml_dtypes.bfloat16
    q = np.asarray(q, dtype=np.float32).astype(bf16)
    k = np.asarray(k, dtype=np.float32).astype(bf16)
    v = np.asarray(v, dtype=np.float32).astype(bf16)
    W_O = np.asarray(W_O, dtype=np.float32).astype(bf16)
    in_maps = []
    for c in range(N_CORES):
        cols = slice(c * CW, (c + 1) * CW)
        in_maps.append(
            {
                "q": np.ascontiguousarray(q[:, :, cols]),
                "k": np.ascontiguousarray(k[:, :, cols]),
                "v": np.ascontiguousarray(v[:, :, cols]),
                "wo": np.ascontiguousarray(
                    W_O[c * HPC : (c + 1) * HPC].reshape(CW, D_MODEL)
                ),
            }
        )
    return in_maps


def get_nc():
    if "nc" not in _CACHE:
        _CACHE["nc"] = _build_bass()
    return _CACHE["nc"]


def kernel(q, k, v, residual, W_O, b_O):
    nc = get_nc()
    in_maps = make_in_maps(q, k, v, W_O)
    res = run_bass_kernel_spmd(nc, in_maps, core_ids=list(range(N_CORES)))
    out = res.results[0]["out"].astype(np.float64)
    for r in res.results[1:]:
        out += r["out"].astype(np.float64)
    out = (out + np.asarray(b_O, dtype=np.float64)[None, None, :]).astype(np.float32)
    return out, np.asarray(residual)
